# revision 3
# baseline (speedup 1.0000x reference)
"""Max pairwise L2 distance between two embedding sets, on 8 Trainium2 cores.

Problem: l [8192, 64] f32, r [8192, 64] f32 -> scalar f32
    out = sqrt(max_ij ||l_i - r_j||^2)

Strategy
--------
The distance matrix has 67M entries; any exact max must examine every one.
On TRN2 the only engines that can read PSUM (where matmul output lands) are
VectorE and ScalarE, so the examination is the bottleneck, not the matmul:

1. On host, pick a strong candidate pair (extreme norms / extreme projections)
   and compute its exact distance L.  Pick thr = L - delta where delta bounds
   the bf16 matmul error.  Any entry <= thr cannot beat L.
2. Augment the K dimension so the PE itself computes sq_dist - thr:
      l_aug = [-2*l | lsq_hi lsq_lo 1 1 1]       (K = 69 rows, bf16)
      r_aug = [  r  | 1 1 rsq_hi rsq_lo -thr]
   (norms carried as bf16 hi+lo pairs for accuracy; thr exactly bf16).
3. Shard rows of l across the 8 cores (1024 each); every core streams all of
   r.  Each core runs 128 matmuls of [69,128]x[69,512] into 2-bank PSUM
   groups.  ScalarE relu+sum-accumulates each group (writing the relu image
   to an SBUF scratch tile, NOT back into PSUM: the in-place PSUM write
   extends the bank-busy window and stalls the PE); VectorE max-reduces any
   remainder groups.  Empirically (HW slope timing) the ScalarE accumulate
   path sustains the PE's full bf16 rate, so ACT takes all groups.
   A partition-row's partial > 0 iff some entry exceeded thr.
4. Host exactly (float64) recomputes the few flagged rows and returns
   sqrt(max(L, flagged maxima)) - an exact fp32 answer.
"""

import numpy as np
import ml_dtypes

N_CORES = 8
N_L, N_R, DIM = 8192, 8192, 64
K_AUG = 69                      # 64 dims + lsq_hi/lo + rsq_hi/lo + thr
L_COLS = N_L // N_CORES         # 1024 l-rows per core
M_TILE = 128                    # stationary free dim (l rows per matmul)
N_FREE = 512                    # moving free dim (one PSUM bank)
CHUNK = 1024                    # consumer group = 2 banks
ACT_FRAC = 0.5                  # fraction of groups consumed by ScalarE
BF16 = ml_dtypes.bfloat16

_COMPILED = {}


def _assignment(groups, act_frac=ACT_FRAC):
    """Bresenham ACT/DVE interleave. Returns list of bools (True = ACT)."""
    n_act = round(groups * act_frac)
    out = []
    acc = 0
    for _ in range(groups):
        acc += n_act
        if acc >= groups:
            acc -= groups
            out.append(True)
        else:
            out.append(False)
    assert sum(out) == n_act
    return out


def _build_nc(l_cols=L_COLS, r_cols=N_R, repeats=1, dyn_loop=False,
              consumer="mixed", chunk=CHUNK, psum_bufs=4,
              act_frac=ACT_FRAC, act_out_sbuf=True):
    """Build + compile the per-core SPMD program.

    Inputs : l_blk [K_AUG, l_cols] bf16, r_all [K_AUG, r_cols] bf16
             (+ rep_cnt [1,1] i32 when dyn_loop)
    Outputs: dve_part [128, n_dve] f32  (max of sq-thr over group)
             act_part [128, n_act] f32  (sum of relu(sq-thr))

    dyn_loop=True wraps the group loop in a runtime-count For_i (for
    timing: one NEFF, variable work).
    """
    import concourse.tile as tile
    from concourse import bacc, mybir
    from concourse.bass import make_scalar_value, RegisterHandles

    m_tiles = l_cols // M_TILE
    n_chunks = r_cols // chunk
    groups = m_tiles * n_chunks
    if consumer == "mixed":
        assign_act = _assignment(groups, act_frac)
    elif consumer == "act":
        assign_act = [True] * groups
    elif consumer in ("dve", "none"):
        assign_act = [False] * groups
    else:
        raise ValueError(consumer)
    n_act = sum(assign_act)
    n_dve = groups - n_act

    nc = bacc.Bacc("TRN2", target_bir_lowering=False, debug=False,
                   num_devices=N_CORES)
    bf16 = mybir.dt.bfloat16
    f32 = mybir.dt.float32

    l_in = nc.dram_tensor("l_blk", [K_AUG, l_cols], bf16,
                          kind="ExternalInput").ap()
    r_in = nc.dram_tensor("r_all", [K_AUG, r_cols], bf16,
                          kind="ExternalInput").ap()
    cnt_in = None
    if dyn_loop:
        cnt_in = nc.dram_tensor("rep_cnt", [1, 1], mybir.dt.int32,
                                kind="ExternalInput").ap()
    dve_out = act_out = None
    if n_dve:
        dve_out = nc.dram_tensor("dve_part", [128, n_dve], f32,
                                 kind="ExternalOutput").ap()
    if n_act:
        act_out = nc.dram_tensor("act_part", [128, n_act], f32,
                                 kind="ExternalOutput").ap()

    with tile.TileContext(nc) as tc:
        with (tc.tile_pool(name="io", bufs=1) as io_pool,
              tc.tile_pool(name="psum", bufs=psum_bufs, space="PSUM") as psum_pool,
              tc.tile_pool(name="scratch", bufs=1) as scratch_pool):
            # tiny dummy activation first so the ACT table set loads during
            # the DMA prologue instead of before the first real group
            warm = scratch_pool.tile([128, 1], f32)
            nc.vector.memset(warm[:], 0.0)
            nc.scalar.activation(warm[:], warm[:],
                                 mybir.ActivationFunctionType.Relu)

            l_sb = io_pool.tile([K_AUG, l_cols], bf16)
            # first m-tiles land first so group 0 starts ASAP
            nc.sync.dma_start(l_sb[:, :2 * M_TILE], l_in[:, :2 * M_TILE])
            nc.sync.dma_start(l_sb[:, 2 * M_TILE:], l_in[:, 2 * M_TILE:])
            r_sb = io_pool.tile([K_AUG, r_cols], bf16)
            for ch in range(n_chunks):
                sl = slice(ch * chunk, (ch + 1) * chunk)
                nc.sync.dma_start(r_sb[:, sl], r_in[:, sl])

            dve_sb = act_sb = None
            if n_dve:
                dve_sb = io_pool.tile([128, n_dve], f32, name="dve_sb")
            if n_act:
                act_sb = io_pool.tile([128, n_act], f32, name="act_sb")
            if consumer == "none" and dve_sb is not None:
                nc.vector.memset(dve_sb[:], 0.0)
            scratch2 = None
            if act_out_sbuf and n_act:
                scratch2 = [scratch_pool.tile([128, chunk], f32,
                                              name=f"relu_img{i}")
                            for i in range(2)]

            def body():
                dve_slot = 0
                act_slot = 0
                # n-chunk outer so the first groups only need r chunk 0
                for g in range(groups):
                    ch, m = divmod(g, m_tiles)
                    ps = psum_pool.tile([128, chunk], f32)
                    for k in range(chunk // N_FREE):
                        ncol = ch * chunk + k * N_FREE
                        nc.tensor.matmul(
                            ps[:, k * N_FREE:(k + 1) * N_FREE],
                            l_sb[:, m * M_TILE:(m + 1) * M_TILE],
                            r_sb[:, ncol:ncol + N_FREE],
                            start=True, stop=True)
                    if consumer == "none":
                        continue
                    if assign_act[g]:
                        # relu image to SBUF scratch (discarded); writing it
                        # back into PSUM keeps the bank busy and stalls PE
                        act_dst = (scratch2[act_slot % 2][:, :]
                                   if act_out_sbuf else ps[:, :])
                        nc.scalar.activation(
                            act_dst, ps[:, :],
                            mybir.ActivationFunctionType.Relu,
                            accum_out=act_sb[:, act_slot:act_slot + 1])
                        act_slot += 1
                    else:
                        nc.vector.tensor_reduce(
                            dve_sb[:, dve_slot:dve_slot + 1], ps[:, :],
                            axis=mybir.AxisListType.X, op=mybir.AluOpType.max)
                        dve_slot += 1

            if dyn_loop:
                cnt_sb = io_pool.tile([1, 1], mybir.dt.int32)
                nc.sync.dma_start(cnt_sb[:], cnt_in[:])
                regs = []
                for etype in mybir.ALL_ENGINES:
                    eng = nc.engines[etype]
                    reg = eng.alloc_register(f"repcnt_{etype.name}")
                    eng.reg_load(reg, cnt_sb[0:1, 0:1])
                    regs.append(reg)
                end_sv = make_scalar_value(
                    RegisterHandles(regs), min_val=0, max_val=100000)
                with tc.For_i(0, end_sv):
                    body()
            else:
                for _ in range(repeats):
                    body()

            if dve_out is not None:
                nc.sync.dma_start(dve_out[:], dve_sb[:])
            if act_out is not None:
                nc.sync.dma_start(act_out[:], act_sb[:])

    nc.compile()
    return nc


def _get_nc(key=("full", 1)):
    if key not in _COMPILED:
        kind, repeats = key
        _COMPILED[key] = _build_nc(repeats=repeats)
    return _COMPILED[key]


def _candidate_threshold(l64, r64, ln, rn):
    """Exact (float64) max squared distance over a cheap candidate set."""
    cl = set(np.argsort(-ln)[:64].tolist())
    cr = set(np.argsort(-rn)[:64].tolist())
    rng = np.random.default_rng(12345)
    U = rng.standard_normal((16, DIM))
    U /= np.linalg.norm(U, axis=1, keepdims=True)
    pl = l64 @ U.T
    pr = r64 @ U.T
    for k in range(U.shape[0]):
        cl.update(np.argsort(-pl[:, k])[:8].tolist())
        cl.update(np.argsort(pl[:, k])[:8].tolist())
        cr.update(np.argsort(-pr[:, k])[:8].tolist())
        cr.update(np.argsort(pr[:, k])[:8].tolist())
    A = l64[sorted(cl)]
    B = r64[sorted(cr)]
    d2 = ((A * A).sum(1)[:, None] + (B * B).sum(1)[None, :]
          - 2.0 * (A @ B.T))
    return float(d2.max())


def _hi_lo_bf16(x64):
    hi = x64.astype(np.float32).astype(BF16)
    lo = (x64 - hi.astype(np.float64)).astype(np.float32).astype(BF16)
    return hi, lo


def _prepare_inputs(l, r):
    """Returns (l_aug [K_AUG, N_L] bf16, r_aug [K_AUG, N_R] bf16, L, thr)."""
    l64 = l.astype(np.float64)
    r64 = r.astype(np.float64)
    lsq = (l64 * l64).sum(1)
    rsq = (r64 * r64).sum(1)
    ln = np.sqrt(lsq)
    rn = np.sqrt(rsq)

    L = _candidate_threshold(l64, r64, ln, rn)
    # bf16 error bound on device sq-dist: cross term 2^-8 * 2*|l||r|, plus
    # slack for norm hi/lo rounding and fp32 accumulation.
    delta = 2.0 ** -8 * 2.0 * float(ln.max()) * float(rn.max()) + 0.05
    thr = float(np.asarray(L - delta, dtype=np.float32).astype(BF16))

    lsq_hi, lsq_lo = _hi_lo_bf16(lsq)
    rsq_hi, rsq_lo = _hi_lo_bf16(rsq)

    l_aug = np.zeros((K_AUG, N_L), dtype=BF16)
    l_aug[:DIM] = (-2.0 * l.astype(np.float32).T).astype(BF16)
    l_aug[64] = lsq_hi
    l_aug[65] = lsq_lo
    l_aug[66] = BF16(1.0)
    l_aug[67] = BF16(1.0)
    l_aug[68] = BF16(1.0)

    r_aug = np.zeros((K_AUG, N_R), dtype=BF16)
    r_aug[:DIM] = r.astype(np.float32).T.astype(BF16)
    r_aug[64] = BF16(1.0)
    r_aug[65] = BF16(1.0)
    r_aug[66] = rsq_hi
    r_aug[67] = rsq_lo
    r_aug[68] = BF16(-thr)

    return np.ascontiguousarray(l_aug), np.ascontiguousarray(r_aug), L, thr


def _run_device(l_aug, r_aug, nc=None):
    from concourse.bass_utils import run_bass_kernel_spmd
    if nc is None:
        nc = _get_nc()
    in_maps = [
        {"l_blk": np.ascontiguousarray(l_aug[:, c * L_COLS:(c + 1) * L_COLS]),
         "r_all": r_aug}
        for c in range(N_CORES)
    ]
    res = run_bass_kernel_spmd(nc, in_maps, core_ids=list(range(N_CORES)))
    return res.results


def kernel(l_dfa_embeddings, r_dfa_embeddings):
    l = np.asarray(l_dfa_embeddings, dtype=np.float32)
    r = np.asarray(r_dfa_embeddings, dtype=np.float32)
    assert l.shape == (N_L, DIM) and r.shape == (N_R, DIM)

    l_aug, r_aug, L, thr = _prepare_inputs(l, r)
    results = _run_device(l_aug, r_aug)

    l64 = l.astype(np.float64)
    r64 = r.astype(np.float64)
    rsq = (r64 * r64).sum(1)

    m_tiles = L_COLS // M_TILE
    groups = m_tiles * (N_R // CHUNK)
    assign_act = _assignment(groups)
    best = L
    for c in range(N_CORES):
        dve = results[c].get("dve_part")
        act = results[c].get("act_part")
        dve_slot = act_slot = 0
        for g in range(groups):
            if assign_act[g]:
                part = act[:, act_slot]
                act_slot += 1
            else:
                part = dve[:, dve_slot]
                dve_slot += 1
            flagged = np.nonzero(part > 0.0)[0]
            if flagged.size == 0:
                continue
            ch, m = divmod(g, m_tiles)
            cols = slice(ch * CHUNK, (ch + 1) * CHUNK)
            for p in flagged:
                lrow = c * L_COLS + m * M_TILE + int(p)
                d2 = ((l64[lrow] * l64[lrow]).sum() + rsq[cols]
                      - 2.0 * (r64[cols] @ l64[lrow]))
                best = max(best, float(d2.max()))

    return np.float32(np.sqrt(max(best, 0.0)))


# revision 4
# speedup vs baseline: 1.3300x; 1.3300x over previous
"""Max pairwise L2 distance, 8 trn2 cores — K=64 row-tiled variant.

Same threshold-matmul scheme as the K=69 version, but the K dimension is
shrunk to 64 (63 data dims + one (rsq - thr) row) so TWO matmuls run
concurrently in PE row-groups 0-63 / 64-127 (measured 3.2x PE, 27% e2e).
The dropped data dim d* and all lsq terms move into host-exact per-row
margins: ScalarE gets them via a per-partition bias AP; the VectorE max
path is compared against per-row thresholds on the host.  Host exactly
(float64) rechecks flagged rows, so margins affect only flag count, never
correctness.
"""

import numpy as np
import ml_dtypes

N_CORES = 8
N_L, N_R, DIM = 8192, 8192, 64
L_COLS = N_L // N_CORES         # 1024 l-rows per core
N_PAIRS = L_COLS // 256         # 4 stationary pairs per core
CHUNK = 1024
N_CHUNKS = N_R // CHUNK
GROUPS = N_PAIRS * N_CHUNKS * 2  # 64 consumer units per core
ACT_FRAC = 0.5
BF16 = ml_dtypes.bfloat16

_COMPILED = {}


def _assignment(groups, act_frac=ACT_FRAC):
    n_act = round(groups * act_frac)
    out = []
    acc = 0
    for _ in range(groups):
        acc += n_act
        if acc >= groups:
            acc -= groups
            out.append(True)
        else:
            out.append(False)
    assert sum(out) == n_act
    return out


def _build_nc(dyn_loop=False, act_frac=ACT_FRAC):
    import concourse.tile as tile
    from concourse import bacc, mybir
    from concourse.bass import make_scalar_value, RegisterHandles

    assign_act = _assignment(GROUPS, act_frac)
    n_act = sum(assign_act)
    n_dve = GROUPS - n_act

    nc = bacc.Bacc("TRN2", target_bir_lowering=False, debug=False,
                   num_devices=N_CORES)
    bf16 = mybir.dt.bfloat16
    f32 = mybir.dt.float32

    l_in = nc.dram_tensor("l_blk", [128, N_PAIRS * 128], bf16,
                          kind="ExternalInput").ap()
    r_in = nc.dram_tensor("r_all", [128, N_R], bf16,
                          kind="ExternalInput").ap()
    b_in = nc.dram_tensor("bias_in", [128, 2 * N_PAIRS], f32,
                          kind="ExternalInput").ap()
    cnt_in = None
    if dyn_loop:
        cnt_in = nc.dram_tensor("rep_cnt", [1, 1], mybir.dt.int32,
                                kind="ExternalInput").ap()
    dve_out = act_out = None
    if n_dve:
        dve_out = nc.dram_tensor("dve_part", [128, n_dve], f32,
                                 kind="ExternalOutput").ap()
    if n_act:
        act_out = nc.dram_tensor("act_part", [128, n_act], f32,
                                 kind="ExternalOutput").ap()

    with tile.TileContext(nc) as tc:
        with (tc.tile_pool(name="io", bufs=1) as io_pool,
              tc.tile_pool(name="psum", bufs=4, space="PSUM") as psum_pool,
              tc.tile_pool(name="scratch", bufs=1) as scratch_pool):
            warm = scratch_pool.tile([128, 1], f32)
            nc.vector.memset(warm[:], 0.0)
            nc.scalar.activation(warm[:], warm[:],
                                 mybir.ActivationFunctionType.Relu)

            l_sb = io_pool.tile([128, N_PAIRS * 128], bf16)
            nc.sync.dma_start(l_sb[:], l_in[:])
            bias_sb = io_pool.tile([128, 2 * N_PAIRS], f32)
            nc.sync.dma_start(bias_sb[:], b_in[:])
            r_sb = io_pool.tile([128, N_R], bf16)
            for ch in range(N_CHUNKS):
                sl = slice(ch * CHUNK, (ch + 1) * CHUNK)
                nc.sync.dma_start(r_sb[:, sl], r_in[:, sl])

            dve_sb = act_sb = None
            if n_dve:
                dve_sb = io_pool.tile([128, n_dve], f32, name="dve_sb")
            if n_act:
                act_sb = io_pool.tile([128, n_act], f32, name="act_sb")
            scratch2 = [scratch_pool.tile([128, CHUNK], f32,
                                          name=f"relu_img{i}")
                        for i in range(2)]

            def consume(g, mt, ps, slots):
                if assign_act[g]:
                    nc.scalar.activation(
                        scratch2[slots[0] % 2][:, :], ps[:, :],
                        mybir.ActivationFunctionType.Relu,
                        bias=bias_sb[:, mt:mt + 1],
                        accum_out=act_sb[:, slots[0]:slots[0] + 1])
                    slots[0] += 1
                else:
                    nc.vector.tensor_reduce(
                        dve_sb[:, slots[1]:slots[1] + 1], ps[:, :],
                        axis=mybir.AxisListType.X, op=mybir.AluOpType.max)
                    slots[1] += 1

            def body():
                slots = [0, 0]
                g = 0
                for ch in range(N_CHUNKS):
                    for t in range(N_PAIRS):
                        psA = psum_pool.tile([128, CHUNK], f32, name="ps",
                                             tag="ps")
                        psB = psum_pool.tile([128, CHUNK], f32, name="ps",
                                             tag="ps")
                        lA = l_sb[0:64, t * 128:(t + 1) * 128]
                        lB = l_sb[64:128, t * 128:(t + 1) * 128]
                        for k in range(2):
                            cols = slice(ch * CHUNK + k * 512,
                                         ch * CHUNK + (k + 1) * 512)
                            seg = slice(k * 512, (k + 1) * 512)
                            nc.tensor.matmul(psA[:, seg], lA,
                                             r_sb[0:64, cols],
                                             start=True, stop=True)
                            nc.tensor.matmul(psB[:, seg], lB,
                                             r_sb[64:128, cols],
                                             start=True, stop=True)
                        consume(g, 2 * t, psA, slots); g += 1
                        consume(g, 2 * t + 1, psB, slots); g += 1

            if dyn_loop:
                cnt_sb = io_pool.tile([1, 1], mybir.dt.int32)
                nc.sync.dma_start(cnt_sb[:], cnt_in[:])
                regs = []
                for etype in mybir.ALL_ENGINES:
                    eng = nc.engines[etype]
                    reg = eng.alloc_register(f"repcnt_{etype.name}")
                    eng.reg_load(reg, cnt_sb[0:1, 0:1])
                    regs.append(reg)
                end_sv = make_scalar_value(
                    RegisterHandles(regs), min_val=0, max_val=100000)
                with tc.For_i(0, end_sv):
                    body()
            else:
                body()

            if dve_out is not None:
                nc.sync.dma_start(dve_out[:], dve_sb[:])
            if act_out is not None:
                nc.sync.dma_start(act_out[:], act_sb[:])

    nc.compile()
    return nc


def _get_nc():
    if "rt" not in _COMPILED:
        _COMPILED["rt"] = _build_nc()
    return _COMPILED["rt"]


def _candidate_threshold(l64, r64, ln, rn):
    cl = set(np.argsort(-ln)[:64].tolist())
    cr = set(np.argsort(-rn)[:64].tolist())
    rng = np.random.default_rng(12345)
    U = rng.standard_normal((16, DIM))
    U /= np.linalg.norm(U, axis=1, keepdims=True)
    pl = l64 @ U.T
    pr = r64 @ U.T
    for k in range(U.shape[0]):
        cl.update(np.argsort(-pl[:, k])[:8].tolist())
        cl.update(np.argsort(pl[:, k])[:8].tolist())
        cr.update(np.argsort(-pr[:, k])[:8].tolist())
        cr.update(np.argsort(pr[:, k])[:8].tolist())
    A = l64[sorted(cl)]
    B = r64[sorted(cr)]
    d2 = ((A * A).sum(1)[:, None] + (B * B).sum(1)[None, :]
          - 2.0 * (A @ B.T))
    return float(d2.max())


def _prepare_inputs(l, r):
    """Returns ((l_pack [128, 512*8], bias [128, 8*8], thresh [N_L]),
    r_dup [128, N_R], L, thr)."""
    l64 = l.astype(np.float64)
    r64 = r.astype(np.float64)
    lsq = (l64 * l64).sum(1)
    rsq = (r64 * r64).sum(1)
    ln = np.sqrt(lsq)
    rn = np.sqrt(rsq)

    L = _candidate_threshold(l64, r64, ln, rn)
    delta_p = 2.0 ** -8 * 2.0 * ln * float(rn.max()) + 0.1
    # dropped dim: minimize worst-case cross-term bound
    dstar = int(np.argmin(np.abs(l64).max(0) * np.abs(r64).max(0)))
    keep = [d for d in range(DIM) if d != dstar]
    w_p = 2.0 * np.abs(l64[:, dstar]) * float(np.abs(r64[:, dstar]).max())
    thr = float(L - 0.05)
    margin = w_p + delta_p + 0.6          # +0.6: bf16 rounding of (rsq-thr)
    bias_rows = (lsq + margin).astype(np.float32)   # [N_L]

    laug = np.zeros((64, N_L), dtype=BF16)
    laug[:63] = (-2.0 * l64[:, keep].T).astype(np.float32).astype(BF16)
    laug[63] = BF16(1.0)
    raug = np.zeros((64, N_R), dtype=BF16)
    raug[:63] = r64[:, keep].T.astype(np.float32).astype(BF16)
    raug[63] = (rsq - thr).astype(np.float32).astype(BF16)

    # pack stationaries: core c, pair t, half h -> l rows c*1024+256t+128h+p
    l_pack = np.zeros((128, N_CORES * N_PAIRS * 128), dtype=BF16)
    bias = np.zeros((128, N_CORES * 2 * N_PAIRS), dtype=np.float32)
    for c in range(N_CORES):
        for t in range(N_PAIRS):
            base = c * L_COLS + 256 * t
            colA = slice(c * 512 + t * 128, c * 512 + (t + 1) * 128)
            l_pack[0:64, colA] = laug[:, base:base + 128]
            l_pack[64:128, colA] = laug[:, base + 128:base + 256]
            bias[:, c * 8 + 2 * t] = bias_rows[base:base + 128]
            bias[:, c * 8 + 2 * t + 1] = bias_rows[base + 128:base + 256]

    r_dup = np.zeros((128, N_R), dtype=BF16)
    r_dup[0:64] = raug
    r_dup[64:128] = raug
    return ((np.ascontiguousarray(l_pack), np.ascontiguousarray(bias),
             bias_rows), np.ascontiguousarray(r_dup), L, thr)


def _per_core_map(laug_obj, r_dup):
    l_pack, bias, _ = laug_obj
    return {
        "l_blk": [np.ascontiguousarray(l_pack[:, c * 512:(c + 1) * 512])
                  for c in range(N_CORES)],
        "r_all": [r_dup for _ in range(N_CORES)],
        "bias_in": [np.ascontiguousarray(bias[:, c * 8:(c + 1) * 8])
                    for c in range(N_CORES)],
    }


def _run_device(laug_obj, r_dup, nc=None):
    from concourse.bass_utils import run_bass_kernel_spmd
    if nc is None:
        nc = _get_nc()
    pc = _per_core_map(laug_obj, r_dup)
    in_maps = [{k: v[c] for k, v in pc.items()} for c in range(N_CORES)]
    res = run_bass_kernel_spmd(nc, in_maps, core_ids=list(range(N_CORES)))
    return res.results


def kernel(l_dfa_embeddings, r_dfa_embeddings):
    l = np.asarray(l_dfa_embeddings, dtype=np.float32)
    r = np.asarray(r_dfa_embeddings, dtype=np.float32)
    assert l.shape == (N_L, DIM) and r.shape == (N_R, DIM)

    laug_obj, r_dup, L, thr = _prepare_inputs(l, r)
    bias_rows = laug_obj[2]
    results = _run_device(laug_obj, r_dup)

    l64 = l.astype(np.float64)
    r64 = r.astype(np.float64)
    rsq = (r64 * r64).sum(1)

    assign_act = _assignment(GROUPS)
    best = L
    for c in range(N_CORES):
        dve = results[c].get("dve_part")
        act = results[c].get("act_part")
        dve_slot = act_slot = 0
        for g in range(GROUPS):
            ch = g // (2 * N_PAIRS)
            t = (g % (2 * N_PAIRS)) // 2
            half = g % 2
            base = c * L_COLS + 256 * t + 128 * half
            if assign_act[g]:
                flagged = np.nonzero(act[:, act_slot] > 0.0)[0]
                act_slot += 1
            else:
                vals = dve[:, dve_slot]
                dve_slot += 1
                flagged = np.nonzero(
                    vals + bias_rows[base:base + 128] > 0.0)[0]
            if flagged.size == 0:
                continue
            cols = slice(ch * CHUNK, (ch + 1) * CHUNK)
            for p in flagged:
                lrow = base + int(p)
                d2 = ((l64[lrow] * l64[lrow]).sum() + rsq[cols]
                      - 2.0 * (r64[cols] @ l64[lrow]))
                best = max(best, float(d2.max()))

    return np.float32(np.sqrt(max(best, 0.0)))
